# revision 8
# baseline (speedup 1.0000x reference)
"""Contrastive-loss kernel for Trainium2 (8 NeuronCores, Bass/Tile).

Math (reference):
    W = wsi[:, 0, :], O = omic[:, 0, :]                      # [N, D]
    S = (W @ O.T) / max(|W_i||O_j|, eps)                     # [N, N] cosine sims
    d = diag(S)
    L = where(eye, 1 - S, relu(M - S + d[:, None]))
    out = mean(L)

Device identity (no diagonal masking of the [N, N] block needed):
    sum(L) = sum_{i,j} relu(hb_i - S_ij) + sum_i [(1 - M - d_i) + (S_ii - d_i)]
    with hb_i = M + d_i (exact, host f64). The fp8 device diagonal S_ii only
    enters through (S_ii - d_i), whose row errors are zero-mean ~2e-3 and sum
    to ~0.1 out of a 1.8e6 grand total (5e-8 relative) - so the device ships
    ONLY the hinge row-sums and the host adds sum_i (1 - M - d_i).

Distribution: data-parallel over W rows. Each core c gets its 512 W rows
(pre-normalized, fp8-e4m3, DoubleRow-packed) plus the full normalized O,
column-rotated by 512*c so the diagonal block always lands in j-chunk 0
(keeps the SPMD program core-independent). Each core computes its
[512, 4096] block of S on the PE (fp8 DoubleRow, fp32 psum) at the fp8
roofline (216 ns / 512-col matmul); the Scalar engine applies the hinge
AND row-sums it in one instruction (activation accum_out), and one
ones-matmul collapses partitions so the output DMA is a single 128-byte
partition line.

Startup schedule: weight/bias DMAs dispatch on the (otherwise idle)
GpSimd queue in parallel with the O-matrix DMAs on the Sync queue, the
first j-chunk is split in half so the first matmul's operands land ~9 us
in, and the PE warmup (two f32 matmuls, 4 cyc/col) is sized to end right
then - an idle PE gap before the real stream would reset the HAM activity
window and hold the clock at 1.2 GHz for an extra window.
"""

import numpy as np
import ml_dtypes

N = 4096
D = 1024
NCORES = 8
ROWS = N // NCORES  # 512 W rows per core
P = 128             # SBUF partitions
NJ = 512            # moving free dim per matmul (one PSUM bank of fp32)
TI = ROWS // P      # 4 i-tiles per core
ND2 = D // 256      # 4 DoubleRow contraction chunks (256 deep each)
NJC = N // NJ       # 8 j-chunks
MARGIN = 0.1
N_WARMUP = 7        # f32 FD=256 PE-warmup matmuls issued while DMAs stream
NCOL = TI * NJC     # one hinge row-sum column per (t, jc) block

_cache = {}


def _build():
    from contextlib import ExitStack
    import concourse.bacc as bacc
    import concourse.tile as tile
    import concourse.mybir as mybir

    f32 = mybir.dt.float32
    bf16 = mybir.dt.bfloat16
    fp8 = mybir.dt.float8e4

    nc = bacc.Bacc("TRN2", target_bir_lowering=False, debug=False,
                   num_devices=NCORES)
    wt_d = nc.dram_tensor("wt", [P, TI * ND2, 2, P], fp8,
                          kind="ExternalInput").ap()
    ot_d = nc.dram_tensor("ot", [P, NJC * ND2, 2, NJ], fp8,
                          kind="ExternalInput").ap()
    hb_d = nc.dram_tensor("hb", [P, TI], f32, kind="ExternalInput").ap()
    out_d = nc.dram_tensor("out", [1, NCOL], f32, kind="ExternalOutput").ap()

    with tile.TileContext(nc) as tc, ExitStack() as ctx:
        const = ctx.enter_context(tc.tile_pool(name="const", bufs=1))
        pp = ctx.enter_context(tc.tile_pool(name="pp", bufs=6, space="PSUM"))
        pp1 = ctx.enter_context(tc.tile_pool(name="pp1", bufs=1, space="PSUM"))
        scrp = ctx.enter_context(tc.tile_pool(name="scr", bufs=4))
        smallp = ctx.enter_context(tc.tile_pool(name="small", bufs=2))

        wt_sb = const.tile([P, TI * ND2, 2, P], fp8, tag="wt")
        ot_sb = const.tile([P, NJC * ND2, 2, NJ], fp8, tag="ot")
        hb = const.tile([P, TI], f32, tag="hb")

        # Three dispatch queues in parallel (all engines share the HW DMA
        # backend, but dispatch serializes per queue): Scalar carries the
        # weight-side tensors, Sync the leading O chunks, GpSimd the rest.
        # Ordered so the first block's operands (wt t0 + first half of
        # j-chunk 0) complete first; later j-chunks stream well ahead of
        # their blocks, coarsening toward the end to cut dispatch +
        # semaphore count.
        nc.scalar.dma_start(out=wt_sb[:, 0:ND2, :, :],
                            in_=wt_d[:, 0:ND2, :, :])
        nc.scalar.dma_start(out=wt_sb[:, ND2:2 * ND2, :, :],
                            in_=wt_d[:, ND2:2 * ND2, :, :])
        nc.scalar.dma_start(out=hb[:], in_=hb_d[:, :])
        nc.scalar.dma_start(out=wt_sb[:, 2 * ND2:, :, :],
                            in_=wt_d[:, 2 * ND2:, :, :])
        nc.sync.dma_start(out=ot_sb[:, 0:ND2 // 2, :, :],
                          in_=ot_d[:, 0:ND2 // 2, :, :])
        nc.sync.dma_start(out=ot_sb[:, ND2 // 2:ND2, :, :],
                          in_=ot_d[:, ND2 // 2:ND2, :, :])
        nc.sync.dma_start(out=ot_sb[:, ND2:2 * ND2, :, :],
                          in_=ot_d[:, ND2:2 * ND2, :, :])
        nc.gpsimd.dma_start(out=ot_sb[:, 2 * ND2:3 * ND2, :, :],
                            in_=ot_d[:, 2 * ND2:3 * ND2, :, :])
        nc.gpsimd.dma_start(out=ot_sb[:, 3 * ND2:4 * ND2, :, :],
                            in_=ot_d[:, 3 * ND2:4 * ND2, :, :])
        nc.gpsimd.dma_start(out=ot_sb[:, 4 * ND2:6 * ND2, :, :],
                            in_=ot_d[:, 4 * ND2:6 * ND2, :, :])
        nc.gpsimd.dma_start(out=ot_sb[:, 6 * ND2:8 * ND2, :, :],
                            in_=ot_d[:, 6 * ND2:8 * ND2, :, :])

        ones_sb = const.tile([P, 1], f32, tag="ones")
        nc.vector.memset(ones_sb[:], 1.0)

        # Warm the PE clock (HAM gate releases after ~3.4us of sustained
        # array activity) while the first DMAs stream, so the real matmul
        # stream starts at 2.4 GHz instead of 1.2 GHz. f32 matmuls stream
        # at 2 cyc/col cold; FD=256 gives ~0.43us granularity so the
        # warmup tail lands close to data-arrival (a PE-idle gap before
        # the real stream resets the HAM activity window).
        warm_rhs = const.tile([P, NJ // 2], f32, tag="warmrhs")
        nc.vector.memset(warm_rhs[:], 0.0)
        warm_ps = pp1.tile([1, NJ // 2], f32, tag="warmps")
        for _ in range(N_WARMUP):
            nc.tensor.matmul(warm_ps[:], lhsT=ones_sb[:], rhs=warm_rhs[:],
                             start=True, stop=True)

        # per-(t,jc) hinge row-sums
        acc = const.tile([P, NCOL], f32, tag="acc")

        for jc in range(NJC):
            for t in range(TI):
                ps = pp.tile([P, NJ], f32, tag="ps")
                for dd in range(ND2):
                    nc.tensor.matmul(
                        ps[:],
                        lhsT=wt_sb[:, t * ND2 + dd, :, :],
                        rhs=ot_sb[:, jc * ND2 + dd, :, :],
                        start=(dd == 0),
                        stop=(dd == ND2 - 1),
                        perf_mode=mybir.MatmulPerfMode.DoubleRow,
                    )
                col = jc * TI + t
                # hinge on ACT (bf16 out halves the DVE reduce cost), row-sum
                # on DVE; each stays under the PE's 864ns block time
                h = scrp.tile([P, NJ], bf16, tag="h")
                nc.scalar.activation(
                    out=h[:],
                    in_=ps[:],
                    func=mybir.ActivationFunctionType.Relu,
                    bias=hb[:, t:t + 1],
                    scale=-1.0,
                )
                nc.vector.tensor_reduce(
                    out=acc[:, col:col + 1], in_=h[:],
                    axis=mybir.AxisListType.X, op=mybir.AluOpType.add)

        # cross-partition reduce on the PE (ones^T @ acc -> [1, 32]) so the
        # output DMA is one contiguous partition line instead of 128 4-byte
        # descriptors (whose completion receipts dominate the kernel tail)
        tot_ps = pp1.tile([1, NCOL], f32, tag="totps")
        nc.tensor.matmul(tot_ps[:], lhsT=ones_sb[:], rhs=acc[:, :],
                         start=True, stop=True)
        total = smallp.tile([1, NCOL], f32, tag="tot")
        nc.vector.tensor_copy(total[:], tot_ps[:])
        nc.sync.dma_start(out=out_d[:, :], in_=total[:])

    nc.compile()
    return nc


def _get_nc():
    if "nc" not in _cache:
        _cache["nc"] = _build()
    return _cache["nc"]


def _prep_inputs(wsi, omic):
    fp8np = ml_dtypes.float8_e4m3
    W = np.asarray(wsi, dtype=np.float32)[:, 0, :].astype(np.float64)
    O = np.asarray(omic, dtype=np.float32)[:, 0, :].astype(np.float64)
    Wn = W / np.maximum(np.linalg.norm(W, axis=1, keepdims=True), 1e-30)
    On = O / np.maximum(np.linalg.norm(O, axis=1, keepdims=True), 1e-30)
    d_exact = np.einsum("nd,nd->n", Wn, On)  # exact cos(w_i, o_i)
    hb_all = (MARGIN + d_exact).astype(np.float32)
    Wn8 = Wn.astype(fp8np)
    On8 = On.astype(fp8np)

    in_maps = []
    for c in range(NCORES):
        Wc = Wn8[c * ROWS:(c + 1) * ROWS]  # [512, 1024]
        # wt[p, t*ND2+dd, r, m] = Wc[t*128+m, dd*256 + r*128 + p]
        wt = np.ascontiguousarray(
            Wc.reshape(TI, P, ND2, 2, P).transpose(4, 0, 2, 3, 1)
            .reshape(P, TI * ND2, 2, P))
        # column rotation: permuted col j' <-> original O row (j' + 512c) % N
        Operm = np.roll(On8, -ROWS * c, axis=0)
        # ot[p, jc*ND2+dd, r, n] = Operm[jc*512 + n, dd*256 + r*128 + p]
        ot = np.ascontiguousarray(
            Operm.reshape(NJC, NJ, ND2, 2, P).transpose(4, 0, 2, 3, 1)
            .reshape(P, NJC * ND2, 2, NJ))
        # hb[p, t] = MARGIN + d_exact[c*512 + t*128 + p]
        hbc = np.ascontiguousarray(
            hb_all[c * ROWS:(c + 1) * ROWS].reshape(TI, P).T)
        in_maps.append({"wt": wt, "ot": ot, "hb": hbc})
    return in_maps, d_exact


def kernel(wsi_embeddings, omic_embeddings):
    from concourse.bass_utils import run_bass_kernel_spmd

    nc = _get_nc()
    in_maps, d_exact = _prep_inputs(wsi_embeddings, omic_embeddings)
    res = run_bass_kernel_spmd(nc, in_maps, list(range(NCORES)))
    # device columns: one relu row-sum per (t, jc) block;
    # host adds the analytic per-row diagonal term sum_i (1 - MARGIN - d_i)
    grand = float(np.sum(1.0 - MARGIN - d_exact))
    for c in range(NCORES):
        grand += res.results[c]["out"].astype(np.float64).sum()
    return np.float32(grand / (float(N) * float(N)))


# revision 12
# speedup vs baseline: 1.0660x; 1.0660x over previous
"""Contrastive-loss kernel for Trainium2 (8 NeuronCores, Bass/Tile).

Math (reference):
    W = wsi[:, 0, :], O = omic[:, 0, :]                      # [N, D]
    S = (W @ O.T) / max(|W_i||O_j|, eps)                     # [N, N] cosine sims
    d = diag(S)
    L = where(eye, 1 - S, relu(M - S + d[:, None]))
    out = mean(L)

Device identity (no diagonal masking of the [N, N] block needed):
    sum(L) = sum_{i,j} relu(hb_i - S_ij) + sum_i [(1 - M - d_i) + (S_ii - d_i)]
    with hb_i = M + d_i (exact, host f64). The fp8 device diagonal S_ii only
    enters through (S_ii - d_i), whose row errors are zero-mean ~2e-3 and sum
    to ~0.1 out of a 1.8e6 grand total (5e-8 relative) - so the device ships
    ONLY the hinge row-sums and the host adds sum_i (1 - M - d_i).

Distribution: data-parallel over W rows. Each core c gets its 512 W rows
(pre-normalized, fp8-e4m3, DoubleRow-packed) plus the full normalized O,
column-rotated by 512*c so the diagonal block always lands in j-chunk 0
(keeps the SPMD program core-independent). Each core computes its
[512, 4096] block of S on the PE (fp8 DoubleRow, fp32 psum) at the fp8
roofline (216 ns / 512-col matmul); the Scalar engine applies the hinge
AND row-sums it in one instruction (activation accum_out), and one
ones-matmul collapses partitions so the output DMA is a single 128-byte
partition line.

Startup schedule: weight/bias DMAs dispatch on the (otherwise idle)
GpSimd queue in parallel with the O-matrix DMAs on the Sync queue, the
first j-chunk is split in half so the first matmul's operands land ~9 us
in, and the PE warmup (two f32 matmuls, 4 cyc/col) is sized to end right
then - an idle PE gap before the real stream would reset the HAM activity
window and hold the clock at 1.2 GHz for an extra window.
"""

import numpy as np
import ml_dtypes

N = 4096
D = 1024
NCORES = 8
ROWS = N // NCORES  # 512 W rows per core
P = 128             # SBUF partitions
NJ = 512            # moving free dim per matmul (one PSUM bank of fp32)
TI = ROWS // P      # 4 i-tiles per core
ND2 = D // 256      # 4 DoubleRow contraction chunks (256 deep each)
NJC = N // NJ       # 8 j-chunks
MARGIN = 0.1
N_WARMUP = 7        # bf16 FD=512 PE-warmup matmuls issued while DMAs stream
NCOL = TI * NJC     # one hinge row-sum column per (t, jc) block

_cache = {}


def _build():
    from contextlib import ExitStack
    import concourse.bacc as bacc
    import concourse.tile as tile
    import concourse.mybir as mybir

    f32 = mybir.dt.float32
    bf16 = mybir.dt.bfloat16
    fp8 = mybir.dt.float8e4

    nc = bacc.Bacc("TRN2", target_bir_lowering=False, debug=False,
                   num_devices=NCORES)
    wt_d = nc.dram_tensor("wt", [P, TI * ND2, 2, P], fp8,
                          kind="ExternalInput").ap()
    ot_d = nc.dram_tensor("ot", [P, NJC * ND2, 2, NJ], fp8,
                          kind="ExternalInput").ap()
    hb_d = nc.dram_tensor("hb", [P, TI], f32, kind="ExternalInput").ap()
    out_d = nc.dram_tensor("out", [1, NCOL], f32, kind="ExternalOutput").ap()

    with tile.TileContext(nc) as tc, ExitStack() as ctx:
        const = ctx.enter_context(tc.tile_pool(name="const", bufs=1))
        pp = ctx.enter_context(tc.tile_pool(name="pp", bufs=6, space="PSUM"))
        pp1 = ctx.enter_context(tc.tile_pool(name="pp1", bufs=1, space="PSUM"))
        scrp = ctx.enter_context(tc.tile_pool(name="scr", bufs=4))
        smallp = ctx.enter_context(tc.tile_pool(name="small", bufs=2))

        wt_sb = const.tile([P, TI * ND2, 2, P], fp8, tag="wt")
        ot_sb = const.tile([P, NJC * ND2, 2, NJ], fp8, tag="ot")
        hb = const.tile([P, TI], f32, tag="hb")

        # Two dispatch queues: Sync carries ALL of O in consumption order
        # (splitting O across queues starves whichever queue gets the
        # smaller engine share and stalls the stream mid-kernel), Scalar
        # carries the small weight-side tensors so their dispatch doesn't
        # delay O's. First block's operands (wt t0 + first half of j-chunk
        # 0) head both queues; trailing chunks coarsen to cut dispatch +
        # semaphore count.
        nc.sync.dma_start(out=ot_sb[:, 0:ND2 // 2, :, :],
                          in_=ot_d[:, 0:ND2 // 2, :, :])
        nc.sync.dma_start(out=ot_sb[:, ND2 // 2:ND2, :, :],
                          in_=ot_d[:, ND2 // 2:ND2, :, :])
        nc.sync.dma_start(out=ot_sb[:, ND2:2 * ND2, :, :],
                          in_=ot_d[:, ND2:2 * ND2, :, :])
        nc.sync.dma_start(out=ot_sb[:, 2 * ND2:3 * ND2, :, :],
                          in_=ot_d[:, 2 * ND2:3 * ND2, :, :])
        nc.sync.dma_start(out=ot_sb[:, 3 * ND2:4 * ND2, :, :],
                          in_=ot_d[:, 3 * ND2:4 * ND2, :, :])
        nc.sync.dma_start(out=ot_sb[:, 4 * ND2:6 * ND2, :, :],
                          in_=ot_d[:, 4 * ND2:6 * ND2, :, :])
        nc.sync.dma_start(out=ot_sb[:, 6 * ND2:8 * ND2, :, :],
                          in_=ot_d[:, 6 * ND2:8 * ND2, :, :])
        nc.scalar.dma_start(out=wt_sb[:, 0:ND2, :, :],
                            in_=wt_d[:, 0:ND2, :, :])
        nc.scalar.dma_start(out=wt_sb[:, ND2:2 * ND2, :, :],
                            in_=wt_d[:, ND2:2 * ND2, :, :])
        nc.scalar.dma_start(out=hb[:], in_=hb_d[:, :])
        nc.scalar.dma_start(out=wt_sb[:, 2 * ND2:, :, :],
                            in_=wt_d[:, 2 * ND2:, :, :])

        ones_sb = const.tile([P, 1], f32, tag="ones")
        nc.vector.memset(ones_sb[:], 1.0)

        # Warm the PE clock (HAM gate releases after ~3.4us of sustained
        # array activity) while the first DMAs stream, so the real matmul
        # stream starts at 2.4 GHz instead of 1.2 GHz. Sized to end at
        # data-arrival (~10us): too short leaves a PE-idle gap that resets
        # the HAM activity window, too long queues ahead of the real
        # stream.
        warm_w = const.tile([P, 1], bf16, tag="warmw")
        nc.vector.memset(warm_w[:], 0.0)
        warm_rhs = const.tile([P, NJ], bf16, tag="warmrhs")
        nc.vector.memset(warm_rhs[:], 0.0)
        warm_ps = pp1.tile([1, NJ], f32, tag="warmps")
        for _ in range(N_WARMUP):
            nc.tensor.matmul(warm_ps[:], lhsT=warm_w[:], rhs=warm_rhs[:],
                             start=True, stop=True)

        # per-(t,jc) hinge row-sums
        acc = const.tile([P, NCOL], f32, tag="acc")

        for jc in range(NJC):
            for t in range(TI):
                ps = pp.tile([P, NJ], f32, tag="ps")
                for dd in range(ND2):
                    nc.tensor.matmul(
                        ps[:],
                        lhsT=wt_sb[:, t * ND2 + dd, :, :],
                        rhs=ot_sb[:, jc * ND2 + dd, :, :],
                        start=(dd == 0),
                        stop=(dd == ND2 - 1),
                        perf_mode=mybir.MatmulPerfMode.DoubleRow,
                    )
                col = jc * TI + t
                # hinge on ACT (bf16 out halves the DVE reduce cost), row-sum
                # on DVE; each stays under the PE's 864ns block time
                h = scrp.tile([P, NJ], bf16, tag="h")
                nc.scalar.activation(
                    out=h[:],
                    in_=ps[:],
                    func=mybir.ActivationFunctionType.Relu,
                    bias=hb[:, t:t + 1],
                    scale=-1.0,
                )
                nc.vector.tensor_reduce(
                    out=acc[:, col:col + 1], in_=h[:],
                    axis=mybir.AxisListType.X, op=mybir.AluOpType.add)

        # cross-partition reduce on the PE (ones^T @ acc -> [1, 32]) so the
        # output DMA is one contiguous partition line instead of 128 4-byte
        # descriptors (whose completion receipts dominate the kernel tail)
        tot_ps = pp1.tile([1, NCOL], f32, tag="totps")
        nc.tensor.matmul(tot_ps[:], lhsT=ones_sb[:], rhs=acc[:, :],
                         start=True, stop=True)
        total = smallp.tile([1, NCOL], f32, tag="tot")
        nc.vector.tensor_copy(total[:], tot_ps[:])
        nc.sync.dma_start(out=out_d[:, :], in_=total[:], single_packet=True)

    nc.compile()
    return nc


def _get_nc():
    if "nc" not in _cache:
        _cache["nc"] = _build()
    return _cache["nc"]


def _prep_inputs(wsi, omic):
    fp8np = ml_dtypes.float8_e4m3
    W = np.asarray(wsi, dtype=np.float32)[:, 0, :].astype(np.float64)
    O = np.asarray(omic, dtype=np.float32)[:, 0, :].astype(np.float64)
    Wn = W / np.maximum(np.linalg.norm(W, axis=1, keepdims=True), 1e-30)
    On = O / np.maximum(np.linalg.norm(O, axis=1, keepdims=True), 1e-30)
    d_exact = np.einsum("nd,nd->n", Wn, On)  # exact cos(w_i, o_i)
    hb_all = (MARGIN + d_exact).astype(np.float32)
    Wn8 = Wn.astype(fp8np)
    On8 = On.astype(fp8np)

    in_maps = []
    for c in range(NCORES):
        Wc = Wn8[c * ROWS:(c + 1) * ROWS]  # [512, 1024]
        # wt[p, t*ND2+dd, r, m] = Wc[t*128+m, dd*256 + r*128 + p]
        wt = np.ascontiguousarray(
            Wc.reshape(TI, P, ND2, 2, P).transpose(4, 0, 2, 3, 1)
            .reshape(P, TI * ND2, 2, P))
        # column rotation: permuted col j' <-> original O row (j' + 512c) % N
        Operm = np.roll(On8, -ROWS * c, axis=0)
        # ot[p, jc*ND2+dd, r, n] = Operm[jc*512 + n, dd*256 + r*128 + p]
        ot = np.ascontiguousarray(
            Operm.reshape(NJC, NJ, ND2, 2, P).transpose(4, 0, 2, 3, 1)
            .reshape(P, NJC * ND2, 2, NJ))
        # hb[p, t] = MARGIN + d_exact[c*512 + t*128 + p]
        hbc = np.ascontiguousarray(
            hb_all[c * ROWS:(c + 1) * ROWS].reshape(TI, P).T)
        in_maps.append({"wt": wt, "ot": ot, "hb": hbc})
    return in_maps, d_exact


def kernel(wsi_embeddings, omic_embeddings):
    from concourse.bass_utils import run_bass_kernel_spmd

    nc = _get_nc()
    in_maps, d_exact = _prep_inputs(wsi_embeddings, omic_embeddings)
    res = run_bass_kernel_spmd(nc, in_maps, list(range(NCORES)))
    # device columns: one relu row-sum per (t, jc) block;
    # host adds the analytic per-row diagonal term sum_i (1 - MARGIN - d_i)
    grand = float(np.sum(1.0 - MARGIN - d_exact))
    for c in range(NCORES):
        grand += res.results[c]["out"].astype(np.float64).sum()
    return np.float32(grand / (float(N) * float(N)))


# revision 15
# speedup vs baseline: 1.1453x; 1.0744x over previous
"""Contrastive-loss kernel for Trainium2 (8 NeuronCores, Bass/Tile).

Math (reference):
    W = wsi[:, 0, :], O = omic[:, 0, :]                      # [N, D]
    S = (W @ O.T) / max(|W_i||O_j|, eps)                     # [N, N] cosine sims
    d = diag(S)
    L = where(eye, 1 - S, relu(M - S + d[:, None]))
    out = mean(L)

Device identity (no diagonal masking of the [N, N] block needed):
    sum(L) = sum_{i,j} relu(hb_i - S_ij) + sum_i [(1 - M - d_i) + (S_ii - d_i)]
    with hb_i = M + d_i (exact, host f64). The fp8 device diagonal S_ii only
    enters through (S_ii - d_i), whose row errors are zero-mean ~2e-3 and sum
    to ~0.1 out of a 1.8e6 grand total (5e-8 relative) - so the device ships
    ONLY the hinge row-sums and the host adds sum_i (1 - M - d_i).

Distribution: data-parallel over W rows. Each core c gets its 512 W rows
(pre-normalized, fp8-e4m3, DoubleRow-packed) plus the full normalized O,
column-rotated by 512*c so the diagonal block always lands in j-chunk 0
(keeps the SPMD program core-independent). Each core computes its
[512, 4096] block of S on the PE (fp8 DoubleRow, fp32 psum) at the fp8
roofline (216 ns / 512-col matmul); the Scalar engine applies the hinge
AND row-sums it in one instruction (activation accum_out), and one
ones-matmul collapses partitions so the output DMA is a single 128-byte
partition line.

Startup schedule: weight/bias DMAs dispatch on the (otherwise idle)
GpSimd queue in parallel with the O-matrix DMAs on the Sync queue, the
first j-chunk is split in half so the first matmul's operands land ~9 us
in, and the PE warmup (two f32 matmuls, 4 cyc/col) is sized to end right
then - an idle PE gap before the real stream would reset the HAM activity
window and hold the clock at 1.2 GHz for an extra window.
"""

import numpy as np
import ml_dtypes

N = 4096
D = 1024
NCORES = 8
ROWS = N // NCORES  # 512 W rows per core
P = 128             # SBUF partitions
NJ = 512            # moving free dim per matmul (one PSUM bank of fp32)
TI = ROWS // P      # 4 i-tiles per core
ND2 = D // 256      # 4 DoubleRow contraction chunks (256 deep each)
NJC = N // NJ       # 8 j-chunks
MARGIN = 0.1
N_WARMUP = 5        # bf16 FD=512 PE-warmup matmuls issued while DMAs stream
NCOL = TI * NJC     # one hinge row-sum column per (t, jc) block

_cache = {}


def _build():
    from contextlib import ExitStack
    import concourse.bacc as bacc
    import concourse.tile as tile
    import concourse.mybir as mybir

    f32 = mybir.dt.float32
    bf16 = mybir.dt.bfloat16
    fp8 = mybir.dt.float8e4

    nc = bacc.Bacc("TRN2", target_bir_lowering=False, debug=False,
                   num_devices=NCORES)
    wt_d = nc.dram_tensor("wt", [P, TI * ND2, 2, P], fp8,
                          kind="ExternalInput").ap()
    ot_d = nc.dram_tensor("ot", [P, NJC * ND2, 2, NJ], fp8,
                          kind="ExternalInput").ap()
    hb_d = nc.dram_tensor("hb", [P, TI], f32, kind="ExternalInput").ap()
    out_d = nc.dram_tensor("out", [1, NCOL], f32, kind="ExternalOutput").ap()

    with tile.TileContext(nc) as tc, ExitStack() as ctx:
        const = ctx.enter_context(tc.tile_pool(name="const", bufs=1))
        pp = ctx.enter_context(tc.tile_pool(name="pp", bufs=6, space="PSUM"))
        pp1 = ctx.enter_context(tc.tile_pool(name="pp1", bufs=1, space="PSUM"))
        scrp = ctx.enter_context(tc.tile_pool(name="scr", bufs=4))
        smallp = ctx.enter_context(tc.tile_pool(name="small", bufs=2))

        wt_sb = const.tile([P, TI * ND2, 2, P], fp8, tag="wt")
        ot_sb = const.tile([P, NJC * ND2, 2, NJ], fp8, tag="ot")
        hb = const.tile([P, TI], f32, tag="hb")

        # ALL input DMAs ride the Sync queue: it is the only queue whose
        # transfers get full DMA-engine bandwidth (Scalar/GpSimd-issued
        # DMAs crawl - measured a 2KB transfer taking 6us there - and a
        # starved queue stalls the stream mid-kernel). Finely split at the
        # head in exact consumption order so the first block's operands
        # land ASAP; trailing chunks coarsen to cut dispatch + semaphore
        # count.
        nc.sync.dma_start(out=wt_sb[:, 0:ND2, :, :],
                          in_=wt_d[:, 0:ND2, :, :])
        nc.sync.dma_start(out=ot_sb[:, 0:1, :, :], in_=ot_d[:, 0:1, :, :])
        nc.sync.dma_start(out=ot_sb[:, 1:2, :, :], in_=ot_d[:, 1:2, :, :])
        nc.sync.dma_start(out=wt_sb[:, ND2:2 * ND2, :, :],
                          in_=wt_d[:, ND2:2 * ND2, :, :])
        nc.sync.dma_start(out=hb[:], in_=hb_d[:, :])
        nc.sync.dma_start(out=ot_sb[:, 2:ND2, :, :], in_=ot_d[:, 2:ND2, :, :])
        nc.sync.dma_start(out=wt_sb[:, 2 * ND2:, :, :],
                          in_=wt_d[:, 2 * ND2:, :, :])
        nc.sync.dma_start(out=ot_sb[:, ND2:2 * ND2, :, :],
                          in_=ot_d[:, ND2:2 * ND2, :, :])
        nc.sync.dma_start(out=ot_sb[:, 2 * ND2:3 * ND2, :, :],
                          in_=ot_d[:, 2 * ND2:3 * ND2, :, :])
        nc.sync.dma_start(out=ot_sb[:, 3 * ND2:4 * ND2, :, :],
                          in_=ot_d[:, 3 * ND2:4 * ND2, :, :])
        nc.sync.dma_start(out=ot_sb[:, 4 * ND2:6 * ND2, :, :],
                          in_=ot_d[:, 4 * ND2:6 * ND2, :, :])
        nc.sync.dma_start(out=ot_sb[:, 6 * ND2:8 * ND2, :, :],
                          in_=ot_d[:, 6 * ND2:8 * ND2, :, :])

        ones_sb = const.tile([P, 1], f32, tag="ones")
        nc.vector.memset(ones_sb[:], 1.0)

        # Warm the PE clock (HAM gate releases after ~3.4us of sustained
        # array activity) while the first DMAs stream, so the real matmul
        # stream starts at 2.4 GHz instead of 1.2 GHz. Sized to end at
        # data-arrival (~10us): too short leaves a PE-idle gap that resets
        # the HAM activity window, too long queues ahead of the real
        # stream.
        warm_w = const.tile([P, 1], bf16, tag="warmw")
        nc.vector.memset(warm_w[:], 0.0)
        warm_rhs = const.tile([P, NJ], bf16, tag="warmrhs")
        nc.vector.memset(warm_rhs[:], 0.0)
        warm_ps = pp1.tile([1, NJ], f32, tag="warmps")
        for _ in range(N_WARMUP):
            nc.tensor.matmul(warm_ps[:], lhsT=warm_w[:], rhs=warm_rhs[:],
                             start=True, stop=True)

        def bridge(n):
            # dummy matmuls woven into the DMA-paced head of the stream:
            # if the next real matmul's data hasn't landed, these keep the
            # PE busy so the HAM activity window doesn't reset (a reset
            # holds the clock at 1.2 GHz for an extra ~3.4us window)
            for _ in range(n):
                nc.tensor.matmul(warm_ps[:, 0:NJ // 2],
                                 lhsT=warm_w[:], rhs=warm_rhs[:, 0:NJ // 2],
                                 start=True, stop=True)

        # per-(t,jc) hinge row-sums
        acc = const.tile([P, NCOL], f32, tag="acc")

        for jc in range(NJC):
            for t in range(TI):
                ps = pp.tile([P, NJ], f32, tag="ps")
                for dd in range(ND2):
                    nc.tensor.matmul(
                        ps[:],
                        lhsT=wt_sb[:, t * ND2 + dd, :, :],
                        rhs=ot_sb[:, jc * ND2 + dd, :, :],
                        start=(dd == 0),
                        stop=(dd == ND2 - 1),
                        perf_mode=mybir.MatmulPerfMode.DoubleRow,
                    )
                    if jc == 0 and t == 0 and dd == 1:
                        bridge(4)
                col = jc * TI + t
                # hinge on ACT (bf16 out halves the DVE reduce cost), row-sum
                # on DVE; each stays under the PE's 864ns block time
                h = scrp.tile([P, NJ], bf16, tag="h")
                nc.scalar.activation(
                    out=h[:],
                    in_=ps[:],
                    func=mybir.ActivationFunctionType.Relu,
                    bias=hb[:, t:t + 1],
                    scale=-1.0,
                )
                nc.vector.tensor_reduce(
                    out=acc[:, col:col + 1], in_=h[:],
                    axis=mybir.AxisListType.X, op=mybir.AluOpType.add)

        # cross-partition reduce on the PE (ones^T @ acc -> [1, 32]) so the
        # output DMA is one contiguous partition line instead of 128 4-byte
        # descriptors (whose completion receipts dominate the kernel tail)
        tot_ps = pp1.tile([1, NCOL], f32, tag="totps")
        nc.tensor.matmul(tot_ps[:], lhsT=ones_sb[:], rhs=acc[:, :],
                         start=True, stop=True)
        total = smallp.tile([1, NCOL], f32, tag="tot")
        nc.vector.tensor_copy(total[:], tot_ps[:])
        nc.sync.dma_start(out=out_d[:, :], in_=total[:], single_packet=True)

    nc.compile()
    return nc


def _get_nc():
    if "nc" not in _cache:
        _cache["nc"] = _build()
    return _cache["nc"]


def _prep_inputs(wsi, omic):
    fp8np = ml_dtypes.float8_e4m3
    W = np.asarray(wsi, dtype=np.float32)[:, 0, :].astype(np.float64)
    O = np.asarray(omic, dtype=np.float32)[:, 0, :].astype(np.float64)
    Wn = W / np.maximum(np.linalg.norm(W, axis=1, keepdims=True), 1e-30)
    On = O / np.maximum(np.linalg.norm(O, axis=1, keepdims=True), 1e-30)
    d_exact = np.einsum("nd,nd->n", Wn, On)  # exact cos(w_i, o_i)
    hb_all = (MARGIN + d_exact).astype(np.float32)
    Wn8 = Wn.astype(fp8np)
    On8 = On.astype(fp8np)

    in_maps = []
    for c in range(NCORES):
        Wc = Wn8[c * ROWS:(c + 1) * ROWS]  # [512, 1024]
        # wt[p, t*ND2+dd, r, m] = Wc[t*128+m, dd*256 + r*128 + p]
        wt = np.ascontiguousarray(
            Wc.reshape(TI, P, ND2, 2, P).transpose(4, 0, 2, 3, 1)
            .reshape(P, TI * ND2, 2, P))
        # column rotation: permuted col j' <-> original O row (j' + 512c) % N
        Operm = np.roll(On8, -ROWS * c, axis=0)
        # ot[p, jc*ND2+dd, r, n] = Operm[jc*512 + n, dd*256 + r*128 + p]
        ot = np.ascontiguousarray(
            Operm.reshape(NJC, NJ, ND2, 2, P).transpose(4, 0, 2, 3, 1)
            .reshape(P, NJC * ND2, 2, NJ))
        # hb[p, t] = MARGIN + d_exact[c*512 + t*128 + p]
        hbc = np.ascontiguousarray(
            hb_all[c * ROWS:(c + 1) * ROWS].reshape(TI, P).T)
        in_maps.append({"wt": wt, "ot": ot, "hb": hbc})
    return in_maps, d_exact


def kernel(wsi_embeddings, omic_embeddings):
    from concourse.bass_utils import run_bass_kernel_spmd

    nc = _get_nc()
    in_maps, d_exact = _prep_inputs(wsi_embeddings, omic_embeddings)
    res = run_bass_kernel_spmd(nc, in_maps, list(range(NCORES)))
    # device columns: one relu row-sum per (t, jc) block;
    # host adds the analytic per-row diagonal term sum_i (1 - MARGIN - d_i)
    grand = float(np.sum(1.0 - MARGIN - d_exact))
    for c in range(NCORES):
        grand += res.results[c]["out"].astype(np.float64).sum()
    return np.float32(grand / (float(N) * float(N)))


# revision 22
# speedup vs baseline: 1.1859x; 1.0355x over previous
"""Contrastive-loss kernel for Trainium2 (8 NeuronCores, Bass/Tile).

Math (reference):
    W = wsi[:, 0, :], O = omic[:, 0, :]                      # [N, D]
    S = (W @ O.T) / max(|W_i||O_j|, eps)                     # [N, N] cosine sims
    d = diag(S)
    L = where(eye, 1 - S, relu(M - S + d[:, None]))
    out = mean(L)

Device identity (no diagonal masking of the [N, N] block needed):
    sum(L) = sum_{i,j} relu(hb_i - S_ij) + sum_i [(1 - M - d_i) + (S_ii - d_i)]
    with hb_i = M + d_i (exact, host f64). The fp8 device diagonal S_ii only
    enters through (S_ii - d_i), whose row errors are zero-mean ~2e-3 and sum
    to ~0.1 out of a 1.8e6 grand total (5e-8 relative) - so the device ships
    ONLY the hinge row-sums and the host adds sum_i (1 - M - d_i).

Distribution: data-parallel over W rows. Each core c gets its 512 W rows
(pre-normalized, fp8-e4m3, DoubleRow-packed) plus the full normalized O,
column-rotated by 512*c so the diagonal block always lands in j-chunk 0
(keeps the SPMD program core-independent). Each core computes its
[512, 4096] block of S on the PE (fp8 DoubleRow, fp32 psum) at the fp8
roofline (216 ns / 512-col matmul); the Scalar engine applies the hinge
AND row-sums it in one instruction (activation accum_out), and one
ones-matmul collapses partitions so the output DMA is a single 128-byte
partition line.

Startup schedule: weight/bias DMAs dispatch on the (otherwise idle)
GpSimd queue in parallel with the O-matrix DMAs on the Sync queue, the
first j-chunk is split in half so the first matmul's operands land ~9 us
in, and the PE warmup (two f32 matmuls, 4 cyc/col) is sized to end right
then - an idle PE gap before the real stream would reset the HAM activity
window and hold the clock at 1.2 GHz for an extra window.
"""

import numpy as np
import ml_dtypes

N = 4096
D = 1024
NCORES = 8
ROWS = N // NCORES  # 512 W rows per core
P = 128             # SBUF partitions
NJ = 512            # moving free dim per matmul (one PSUM bank of fp32)
TI = ROWS // P      # 4 i-tiles per core
ND2 = D // 256      # 4 DoubleRow contraction chunks (256 deep each)
NJC = N // NJ       # 8 j-chunks
MARGIN = 0.1
N_WARMUP = 8        # bf16 FD=512 PE-warmup matmuls issued while DMAs stream
NCOL = TI * NJC     # one hinge row-sum column per (t, jc) block

_cache = {}


def _build():
    from contextlib import ExitStack
    import concourse.bacc as bacc
    import concourse.tile as tile
    import concourse.mybir as mybir

    f32 = mybir.dt.float32
    bf16 = mybir.dt.bfloat16
    fp8 = mybir.dt.float8e4

    nc = bacc.Bacc("TRN2", target_bir_lowering=False, debug=False,
                   num_devices=NCORES)
    wt_d = nc.dram_tensor("wt", [P, TI * ND2, 2, P], fp8,
                          kind="ExternalInput").ap()
    ot_d = nc.dram_tensor("ot", [P, NJC * ND2, 2, NJ], fp8,
                          kind="ExternalInput").ap()
    hb_d = nc.dram_tensor("hb", [P, TI], f32, kind="ExternalInput").ap()
    out_d = nc.dram_tensor("out", [1, NCOL], f32, kind="ExternalOutput").ap()

    with tile.TileContext(nc) as tc, ExitStack() as ctx:
        const = ctx.enter_context(tc.tile_pool(name="const", bufs=1))
        pp = ctx.enter_context(tc.tile_pool(name="pp", bufs=6, space="PSUM"))
        pp1 = ctx.enter_context(tc.tile_pool(name="pp1", bufs=1, space="PSUM"))
        scrp = ctx.enter_context(tc.tile_pool(name="scr", bufs=4))
        smallp = ctx.enter_context(tc.tile_pool(name="small", bufs=2))

        wt_sb = const.tile([P, TI * ND2, 2, P], fp8, tag="wt")
        ot_sb = const.tile([P, NJC * ND2, 2, NJ], fp8, tag="ot")
        hb = const.tile([P, TI], f32, tag="hb")

        # ALL input DMAs ride the Sync queue: it is the only queue whose
        # transfers get full DMA-engine bandwidth (Scalar/GpSimd-issued
        # DMAs crawl - measured a 2KB transfer taking 6us there - and a
        # starved queue stalls the stream mid-kernel). Finely split at the
        # head in exact consumption order so the first block's operands
        # land ASAP; trailing chunks coarsen to cut dispatch + semaphore
        # count.
        nc.sync.dma_start(out=wt_sb[:, 0:ND2, :, :],
                          in_=wt_d[:, 0:ND2, :, :])
        nc.sync.dma_start(out=ot_sb[:, 0:1, :, :], in_=ot_d[:, 0:1, :, :])
        nc.sync.dma_start(out=ot_sb[:, 1:2, :, :], in_=ot_d[:, 1:2, :, :])
        nc.sync.dma_start(out=wt_sb[:, ND2:2 * ND2, :, :],
                          in_=wt_d[:, ND2:2 * ND2, :, :])
        nc.sync.dma_start(out=ot_sb[:, 2:ND2, :, :], in_=ot_d[:, 2:ND2, :, :])
        nc.sync.dma_start(out=hb[:], in_=hb_d[:, :])
        nc.sync.dma_start(out=wt_sb[:, 2 * ND2:, :, :],
                          in_=wt_d[:, 2 * ND2:, :, :])
        nc.sync.dma_start(out=ot_sb[:, ND2:2 * ND2, :, :],
                          in_=ot_d[:, ND2:2 * ND2, :, :])
        nc.sync.dma_start(out=ot_sb[:, 2 * ND2:3 * ND2, :, :],
                          in_=ot_d[:, 2 * ND2:3 * ND2, :, :])
        nc.sync.dma_start(out=ot_sb[:, 3 * ND2:4 * ND2, :, :],
                          in_=ot_d[:, 3 * ND2:4 * ND2, :, :])
        nc.sync.dma_start(out=ot_sb[:, 4 * ND2:6 * ND2, :, :],
                          in_=ot_d[:, 4 * ND2:6 * ND2, :, :])
        nc.sync.dma_start(out=ot_sb[:, 6 * ND2:8 * ND2, :, :],
                          in_=ot_d[:, 6 * ND2:8 * ND2, :, :])

        # Warm the PE clock (HAM gate releases after ~3.4us of sustained
        # array activity) while the first DMAs stream, so the real matmul
        # stream starts at 2.4 GHz instead of 1.2 GHz. Operands come from
        # the framework const pool (memset during the preamble), so the
        # first warmup issues the moment the Tensor engine clears the
        # startup barrier - no user memset in front.
        ones_c = nc.const_aps.tensor(1.0, (P, 1), f32)
        warm_w = nc.const_aps.tensor(1.0, (P, 1), bf16)
        warm_rhs = nc.const_aps.tensor(1.0, (P, NJ), bf16)
        warm_ps = pp1.tile([1, NJ], f32, tag="warmps")
        for _ in range(N_WARMUP):
            nc.tensor.matmul(warm_ps[:], lhsT=warm_w, rhs=warm_rhs,
                             start=True, stop=True)

        def bridge(n):
            # dummy matmuls woven into the DMA-paced head of the stream:
            # if the next real matmul's data hasn't landed, these keep the
            # PE busy so the HAM activity window doesn't reset (a reset
            # holds the clock at 1.2 GHz for an extra ~3.4us window)
            for _ in range(n):
                nc.tensor.matmul(warm_ps[:, 0:NJ // 2],
                                 lhsT=warm_w, rhs=warm_rhs[:, 0:NJ // 2],
                                 start=True, stop=True)

        # per-(t,jc) hinge row-sums
        acc = const.tile([P, NCOL], f32, tag="acc")

        bridge(5)  # fine-grained cover between warmup end and first data
        for jc in range(NJC):
            for t in range(TI):
                ps = pp.tile([P, NJ], f32, tag="ps")
                for dd in range(ND2):
                    nc.tensor.matmul(
                        ps[:],
                        lhsT=wt_sb[:, t * ND2 + dd, :, :],
                        rhs=ot_sb[:, jc * ND2 + dd, :, :],
                        start=(dd == 0),
                        stop=(dd == ND2 - 1),
                        perf_mode=mybir.MatmulPerfMode.DoubleRow,
                    )
                    if jc == 0 and t == 0 and dd == 0:
                        bridge(2)
                    if jc == 0 and t == 0 and dd == 1:
                        bridge(2)
                col = jc * TI + t
                # hinge on ACT (bf16 out halves the DVE reduce cost), row-sum
                # on DVE; each stays under the PE's 864ns block time
                h = scrp.tile([P, NJ], bf16, tag="h")
                nc.scalar.activation(
                    out=h[:],
                    in_=ps[:],
                    func=mybir.ActivationFunctionType.Relu,
                    bias=hb[:, t:t + 1],
                    scale=-1.0,
                )
                nc.vector.tensor_reduce(
                    out=acc[:, col:col + 1], in_=h[:],
                    axis=mybir.AxisListType.X, op=mybir.AluOpType.add)

        # cross-partition reduce on the PE (ones^T @ acc -> [1, 32]) so the
        # output DMA is one contiguous partition line instead of 128 4-byte
        # descriptors (whose completion receipts dominate the kernel tail)
        tot_ps = pp1.tile([1, NCOL], f32, tag="totps")
        nc.tensor.matmul(tot_ps[:], lhsT=ones_c, rhs=acc[:, :],
                         start=True, stop=True)
        total = smallp.tile([1, NCOL], f32, tag="tot")
        nc.vector.tensor_copy(total[:], tot_ps[:])
        nc.sync.dma_start(out=out_d[:, :], in_=total[:], single_packet=True)

    nc.compile()
    return nc


def _get_nc():
    if "nc" not in _cache:
        _cache["nc"] = _build()
    return _cache["nc"]


def _prep_inputs(wsi, omic):
    fp8np = ml_dtypes.float8_e4m3
    W = np.asarray(wsi, dtype=np.float32)[:, 0, :].astype(np.float64)
    O = np.asarray(omic, dtype=np.float32)[:, 0, :].astype(np.float64)
    Wn = W / np.maximum(np.linalg.norm(W, axis=1, keepdims=True), 1e-30)
    On = O / np.maximum(np.linalg.norm(O, axis=1, keepdims=True), 1e-30)
    d_exact = np.einsum("nd,nd->n", Wn, On)  # exact cos(w_i, o_i)
    hb_all = (MARGIN + d_exact).astype(np.float32)
    Wn8 = Wn.astype(fp8np)
    On8 = On.astype(fp8np)

    in_maps = []
    for c in range(NCORES):
        Wc = Wn8[c * ROWS:(c + 1) * ROWS]  # [512, 1024]
        # wt[p, t*ND2+dd, r, m] = Wc[t*128+m, dd*256 + r*128 + p]
        wt = np.ascontiguousarray(
            Wc.reshape(TI, P, ND2, 2, P).transpose(4, 0, 2, 3, 1)
            .reshape(P, TI * ND2, 2, P))
        # column rotation: permuted col j' <-> original O row (j' + 512c) % N
        Operm = np.roll(On8, -ROWS * c, axis=0)
        # ot[p, jc*ND2+dd, r, n] = Operm[jc*512 + n, dd*256 + r*128 + p]
        ot = np.ascontiguousarray(
            Operm.reshape(NJC, NJ, ND2, 2, P).transpose(4, 0, 2, 3, 1)
            .reshape(P, NJC * ND2, 2, NJ))
        # hb[p, t] = MARGIN + d_exact[c*512 + t*128 + p]
        hbc = np.ascontiguousarray(
            hb_all[c * ROWS:(c + 1) * ROWS].reshape(TI, P).T)
        in_maps.append({"wt": wt, "ot": ot, "hb": hbc})
    return in_maps, d_exact


def kernel(wsi_embeddings, omic_embeddings):
    from concourse.bass_utils import run_bass_kernel_spmd

    nc = _get_nc()
    in_maps, d_exact = _prep_inputs(wsi_embeddings, omic_embeddings)
    res = run_bass_kernel_spmd(nc, in_maps, list(range(NCORES)))
    # device columns: one relu row-sum per (t, jc) block;
    # host adds the analytic per-row diagonal term sum_i (1 - MARGIN - d_i)
    grand = float(np.sum(1.0 - MARGIN - d_exact))
    for c in range(NCORES):
        grand += res.results[c]["out"].astype(np.float64).sum()
    return np.float32(grand / (float(N) * float(N)))
